# revision 1
# baseline (speedup 1.0000x reference)
"""GCN layer (Linear -> weighted-adjacency SpMM -> BatchNorm(eval) -> exact GELU)
as a Bass/Tile kernel on 8 Trainium2 NeuronCores.

Sharding: destination-node rows are sharded across the 8 cores (12500 rows each);
edges are partitioned by destination row.  W/b/BN params are replicated.  Each
core computes the full `support = x @ W' + b'` redundantly (cheaper than
collectives here), writing it as 4 row-sections so the phase-2 gathers of
section s only depend on section s (overlaps SpMM with the tail of the matmul).

Phase 2 per destination tile (128 rows): source rows of `support` are fetched
with one big `dma_gather` (int16 section-local indices, runtime valid-count
register, negative-index tail padding), and segment-summed on the tensor engine
with per-128-edge-chunk one-hot selector matmuls accumulating in PSUM; partial
sums per section accumulate into an SBUF-resident accumulator.  BN is folded on
the host (W' = W * s, shift = beta - mean * s, s = gamma / sqrt(var + eps)), so
the epilogue is one add + one exact-GELU activation per tile.

Host-side prep inside kernel(): transpose x; per core group edges by
(source-section, destination-tile); pack per-group edge row/val into fixed
128-edge chunk layout and indices into the 16-partition-wrapped int16 layout
dma_gather expects.  One SPMD program serves all 8 cores; per-group edge counts
ride in as data (the count register trims descriptor generation to real edges).
"""

import sys

sys.path.insert(0, "/opt/trn_rl_repo")

import numpy as np

import concourse.tile as tile
from concourse import bacc, mybir
from concourse.bass_utils import run_bass_kernel_spmd

F32 = mybir.dt.float32
I32 = mybir.dt.int32
I16 = mybir.dt.int16
AF = mybir.ActivationFunctionType
ALU = mybir.AluOpType

N_CORES = 8
TPS = 14      # dest tiles per index-slab load (must divide nt)
XCOLS = 512   # node columns per phase-1 supertile
NGBUF = 5     # round-robin gather buffers


def _build_program(*, in_dim, out_dim, npad, nt, c_sub, tps, xcols):
    assert in_dim % 128 == 0 and npad % (4 * xcols) == 0
    assert nt % tps == 0
    kb = in_dim // 128
    sec_rows = npad // 4
    nsup_sec = sec_rows // xcols
    jt = xcols // 128
    nidx = c_sub * 128
    idxcols = nidx // 16

    nc = bacc.Bacc("TRN2", target_bir_lowering=False, debug=False,
                   num_devices=N_CORES)

    xT = nc.dram_tensor("xT", [in_dim, npad], F32, kind="ExternalInput").ap()
    Wp = nc.dram_tensor("Wp", [in_dim, out_dim], F32, kind="ExternalInput").ap()
    bp = nc.dram_tensor("bp", [128, out_dim], F32, kind="ExternalInput").ap()
    shiftb = nc.dram_tensor("shiftb", [128, out_dim], F32, kind="ExternalInput").ap()
    iota_in = nc.dram_tensor("iota", [128, 128], F32, kind="ExternalInput").ap()
    idxp = nc.dram_tensor("idxp", [128, 4 * nt * idxcols], I16,
                          kind="ExternalInput").ap()
    rowp = nc.dram_tensor("rowp", [128, 4 * nt * c_sub], F32,
                          kind="ExternalInput").ap()
    valp = nc.dram_tensor("valp", [128, 4 * nt * c_sub], F32,
                          kind="ExternalInput").ap()
    cnts = nc.dram_tensor("cnts", [1, 4 * nt], I32, kind="ExternalInput").ap()
    out = nc.dram_tensor("out", [nt * 128, out_dim], F32, kind="ExternalOutput").ap()
    secs = [nc.dram_tensor(f"support{s}", [sec_rows, out_dim], F32).ap()
            for s in range(4)]

    with tile.TileContext(nc) as tc, tc.tile_pool(name="consts", bufs=1) as consts:
        w_sb = consts.tile([128, kb, out_dim], F32)
        bp_sb = consts.tile([128, out_dim], F32)
        shift_sb = consts.tile([128, out_dim], F32)
        iota_sb = consts.tile([128, 128], F32)
        cnt_sb = consts.tile([128, 4 * nt], I32)
        acc = consts.tile([128, nt, out_dim], F32)
        gts = consts.tile([128, NGBUF, c_sub, out_dim], F32)
        for i in range(NGBUF):
            nc.vector.memset(gts[:, i], 0.0)
        for k in range(kb):
            nc.sync.dma_start(w_sb[:, k, :], Wp[k * 128:(k + 1) * 128, :])
        nc.sync.dma_start(bp_sb[:], bp[:])
        nc.sync.dma_start(shift_sb[:], shiftb[:])
        nc.sync.dma_start(iota_sb[:], iota_in[:])
        nc.sync.dma_start(cnt_sb[0:1, :], cnts[:])

        # Phase-2 pools opened first: disjoint SBUF from phase-1 pools, so
        # phase-2 allocations carry no WAR deps on phase-1 releases.
        with (
            tc.tile_pool(name="slabs", bufs=2) as slabs,
            tc.tile_pool(name="sel", bufs=2) as selpool,
            tc.tile_pool(name="p2psum", bufs=6, space="PSUM") as p2psum,
        ):
            with (
                tc.tile_pool(name="xt", bufs=2) as xpool,
                tc.tile_pool(name="p1psum", bufs=2, space="PSUM") as p1psum,
                tc.tile_pool(name="p1out", bufs=4) as p1out,
            ):
                def p1_section(s4):
                    for st in range(nsup_sec):
                        gcol = (s4 * nsup_sec + st) * xcols
                        xt = xpool.tile([128, kb, xcols], F32)
                        for k in range(kb):
                            nc.sync.dma_start(
                                xt[:, k, :],
                                xT[k * 128:(k + 1) * 128, gcol:gcol + xcols])
                        for j in range(jt):
                            ps = p1psum.tile([128, out_dim], F32)
                            for k in range(kb):
                                nc.tensor.matmul(
                                    ps[:], lhsT=xt[:, k, j * 128:(j + 1) * 128],
                                    rhs=w_sb[:, k, :],
                                    start=(k == 0), stop=(k == kb - 1))
                            so = p1out.tile([128, out_dim], F32)
                            nc.vector.tensor_tensor(so[:], ps[:], bp_sb[:],
                                                    op=ALU.add)
                            r0 = (st * jt + j) * 128
                            nc.sync.dma_start(secs[s4][r0:r0 + 128, :], so[:])

                nreg = nc.gpsimd.alloc_register("gcnt")
                gbuf_i = 0

                def p2_pass(s):
                    nonlocal gbuf_i
                    for sl in range(nt // tps):
                            idx_sb = slabs.tile([128, tps * idxcols], I16, tag="idx")
                            row_sb = slabs.tile([128, tps * c_sub], F32, tag="row")
                            val_sb = slabs.tile([128, tps * c_sub], F32, tag="val")
                            gbase = s * nt + sl * tps
                            nc.sync.dma_start(
                                idx_sb[:], idxp[:, gbase * idxcols:(gbase + tps) * idxcols])
                            nc.sync.dma_start(
                                row_sb[:], rowp[:, gbase * c_sub:(gbase + tps) * c_sub])
                            nc.sync.dma_start(
                                val_sb[:], valp[:, gbase * c_sub:(gbase + tps) * c_sub])
                            for tt in range(tps):
                                t = sl * tps + tt
                                g = s * nt + t
                                # sel[p, c, d] = (row[p, c] == d) * val[p, c]
                                sel = selpool.tile([128, c_sub, 128], F32)
                                row3 = row_sb[:, tt * c_sub:(tt + 1) * c_sub].unsqueeze(2) \
                                    .to_broadcast([128, c_sub, 128])
                                val3 = val_sb[:, tt * c_sub:(tt + 1) * c_sub].unsqueeze(2) \
                                    .to_broadcast([128, c_sub, 128])
                                iota3 = iota_sb[:].unsqueeze(1) \
                                    .to_broadcast([128, c_sub, 128])
                                nc.vector.tensor_tensor(sel[:], row3, iota3, op=ALU.is_equal)
                                nc.vector.tensor_tensor(sel[:], sel[:], val3, op=ALU.mult)
                                gt = gts[:, gbuf_i % NGBUF]
                                gbuf_i += 1
                                nc.gpsimd.reg_load(nreg, cnt_sb[0:1, g:g + 1])
                                nc.gpsimd.dma_gather(
                                    out_ap=gt[:],
                                    in_ap=secs[s][:],
                                    idxs_ap=idx_sb[:, tt * idxcols:(tt + 1) * idxcols],
                                    num_idxs=nidx,
                                    num_idxs_reg=nreg,
                                    elem_size=out_dim,
                                    single_packet=False,
                                )
                                ps = p2psum.tile([128, out_dim], F32)
                                for u in range(c_sub):
                                    nc.tensor.matmul(
                                        ps[:], lhsT=sel[:, u, :], rhs=gt[:, u, :],
                                        start=(u == 0), stop=(u == c_sub - 1))
                                if s == 0:
                                    nc.vector.tensor_copy(acc[:, t, :], ps[:])
                                elif s < 3:
                                    nc.vector.tensor_tensor(acc[:, t, :], acc[:, t, :],
                                                            ps[:], op=ALU.add)
                                else:
                                    # final section: fuse BN shift + GELU + store
                                    ob = selpool.tile([128, out_dim], F32, tag="ob")
                                    nc.vector.tensor_tensor(ob[:], acc[:, t, :],
                                                            ps[:], op=ALU.add)
                                    ob2 = selpool.tile([128, out_dim], F32, tag="ob2")
                                    nc.vector.tensor_tensor(ob2[:], ob[:],
                                                            shift_sb[:], op=ALU.add)
                                    ob3 = selpool.tile([128, out_dim], F32, tag="ob3")
                                    nc.scalar.activation(ob3[:], ob2[:], AF.Gelu)
                                    nc.sync.dma_start(out[t * 128:(t + 1) * 128, :],
                                                      ob3[:])

                p1_section(0)
                p1_section(1)
                p2_pass(0)
                p1_section(2)
                p2_pass(1)
                p1_section(3)
                p2_pass(2)
                p2_pass(3)

    nc.compile()
    return nc


def _preprocess(x, edge_row, edge_col, edge_val, W, b, gamma, beta,
                running_mean, running_var, bn_eps=1e-5):
    n, in_dim = x.shape
    out_dim = W.shape[1]
    npad = ((n + 4 * XCOLS - 1) // (4 * XCOLS)) * (4 * XCOLS)
    sec_rows = npad // 4
    assert sec_rows <= 32768, "support section must be int16-addressable"
    shard = n // N_CORES
    assert shard * N_CORES == n
    nt = (shard + 127) // 128
    nt = ((nt + TPS - 1) // TPS) * TPS

    inv_std = 1.0 / np.sqrt(running_var.astype(np.float64) + bn_eps)
    scale = (inv_std * gamma.astype(np.float64)).astype(np.float32)
    shift = (beta.astype(np.float64) - running_mean.astype(np.float64) * inv_std
             * gamma.astype(np.float64)).astype(np.float32)

    xT = np.zeros((in_dim, npad), np.float32)
    xT[:, :n] = np.ascontiguousarray(x.T)
    Wp = (W * scale[None, :]).astype(np.float32)
    bp = np.ascontiguousarray(
        np.broadcast_to((b * scale).astype(np.float32), (128, out_dim)))
    shiftb = np.ascontiguousarray(np.broadcast_to(shift, (128, out_dim)))
    iota = np.ascontiguousarray(
        np.broadcast_to(np.arange(128, dtype=np.float32), (128, 128)))

    per_core = []
    c_sub = 1
    ng = 4 * nt
    for m in range(N_CORES):
        lo, hi = m * shard, (m + 1) * shard
        mask = (edge_row >= lo) & (edge_row < hi)
        er = (edge_row[mask] - lo).astype(np.int64)
        ec = edge_col[mask].astype(np.int64)
        ev = edge_val[mask].astype(np.float32)
        tile_of = er >> 7
        sec_of = ec // sec_rows
        gid = sec_of * nt + tile_of
        order = np.argsort(gid, kind="stable")
        er, ec, ev, gid = er[order], ec[order], ev[order], gid[order]
        counts = np.bincount(gid, minlength=ng)
        per_core.append((er, ec, ev, gid, counts))
        c_sub = max(c_sub, int(((counts + 127) // 128).max()))
    nidx = c_sub * 128
    idxcols = nidx // 16

    in_maps = []
    for m in range(N_CORES):
        er, ec, ev, gid, counts = per_core[m]
        starts = np.zeros(ng, np.int64)
        np.cumsum(counts[:-1], out=starts[1:])
        rank = np.arange(len(er)) - starts[gid]
        rowp = np.zeros((128, ng * c_sub), np.float32)
        valp = np.zeros((128, ng * c_sub), np.float32)
        rowp[rank & 127, gid * c_sub + (rank >> 7)] = (er & 127).astype(np.float32)
        valp[rank & 127, gid * c_sub + (rank >> 7)] = ev
        idx16 = np.full((16, ng * idxcols), -1, np.int16)
        idx16[rank & 15, gid * idxcols + (rank >> 4)] = \
            (ec % sec_rows).astype(np.int16)
        cnts_arr = counts.astype(np.int32)
        empty = np.nonzero(cnts_arr == 0)[0]
        if len(empty):
            idx16[0, empty * idxcols] = 0  # one dummy valid index, val stays 0
            cnts_arr[empty] = 1
        in_maps.append({
            "xT": xT, "Wp": Wp, "bp": bp, "shiftb": shiftb, "iota": iota,
            "idxp": np.ascontiguousarray(np.tile(idx16, (8, 1))),
            "rowp": np.ascontiguousarray(rowp),
            "valp": np.ascontiguousarray(valp),
            "cnts": cnts_arr.reshape(1, ng),
        })

    params = dict(in_dim=in_dim, out_dim=out_dim, npad=npad,
                  nt=nt, c_sub=c_sub, tps=TPS, xcols=XCOLS)
    return in_maps, params, shard


def kernel(x, edge_row, edge_col, edge_val, W, b, gamma, beta,
           running_mean, running_var):
    x = np.asarray(x)
    edge_row = np.asarray(edge_row)
    edge_col = np.asarray(edge_col)
    edge_val = np.asarray(edge_val)
    W = np.asarray(W)
    b = np.asarray(b)
    gamma = np.asarray(gamma)
    beta = np.asarray(beta)
    running_mean = np.asarray(running_mean)
    running_var = np.asarray(running_var)

    in_maps, params, shard = _preprocess(
        x, edge_row, edge_col, edge_val, W, b, gamma, beta,
        running_mean, running_var)
    nc = _build_program(**params)
    res = run_bass_kernel_spmd(nc, in_maps, core_ids=list(range(N_CORES)))
    outs = [res.results[m]["out"][:shard] for m in range(N_CORES)]
    return np.concatenate(outs, axis=0).astype(np.float32)



# revision 2
# speedup vs baseline: 2.9901x; 2.9901x over previous
"""GCN layer (Linear -> weighted-adjacency SpMM -> BatchNorm(eval) -> exact GELU)
as a Bass/Tile kernel on 8 Trainium2 NeuronCores.

Sharding: destination-node rows are sharded across the 8 cores (12500 rows each);
edges are partitioned by destination row.  W/b/BN params are replicated.  Each
core computes the full `support = x @ W' + b'` redundantly (cheaper than
collectives here), writing it as 4 row-sections in bf16 so the phase-2 gathers
of section s only depend on section s (overlaps SpMM with the tail of the
matmul).

The whole data path is bf16 (x, W, support, gathered rows, selectors); only
PSUM accumulation, the cross-section accumulator, bias/shift, and the final
GELU/output stay f32.  bf16 halves HBM gather traffic (512B per row), doubles
PE matmul rate and DVE selector-build rate.

Phase 2 per destination tile (128 rows): source rows of `support` are fetched
with one big `dma_gather` (int16 section-local indices, constant full index
count - per-group padding points at row 0 with val 0, so no per-gather count
register load), and segment-summed on the tensor engine with per-128-edge-chunk
one-hot selector matmuls accumulating in PSUM; partial sums per section
accumulate into an SBUF-resident f32 accumulator.  BN is folded on the host
(W' = W * s, shift = beta - mean * s, s = gamma / sqrt(var + eps)), so the
epilogue is one add + one exact-GELU activation per tile.

Host-side prep inside kernel(): transpose x; per core group edges by
(source-section, destination-tile); pack per-group edge row/val into fixed
128-edge chunk layout and indices into the 16-partition-wrapped int16 layout
dma_gather expects.  One SPMD program serves all 8 cores.
"""

import sys

sys.path.insert(0, "/opt/trn_rl_repo")

import numpy as np
import ml_dtypes

import concourse.tile as tile
from concourse import bacc, mybir
from concourse.bass_utils import run_bass_kernel_spmd

F32 = mybir.dt.float32
BF16 = mybir.dt.bfloat16
I32 = mybir.dt.int32
I16 = mybir.dt.int16
AF = mybir.ActivationFunctionType
ALU = mybir.AluOpType

BF = ml_dtypes.bfloat16

N_CORES = 8
TPS = 14      # dest tiles per index-slab load (must divide nt)
XCOLS = 512   # node columns per phase-1 supertile
NGBUF = 6     # round-robin gather buffers


def _build_program(*, in_dim, out_dim, npad, nt, c_sub, tps, xcols):
    assert in_dim % 128 == 0 and npad % (4 * xcols) == 0
    assert nt % tps == 0
    kb = in_dim // 128
    sec_rows = npad // 4
    nsup_sec = sec_rows // xcols
    jt = xcols // 128
    nidx = c_sub * 128
    idxcols = nidx // 16

    nc = bacc.Bacc("TRN2", target_bir_lowering=False, debug=False,
                   num_devices=N_CORES)

    xT = nc.dram_tensor("xT", [in_dim, npad], BF16, kind="ExternalInput").ap()
    Wp = nc.dram_tensor("Wp", [in_dim, out_dim], BF16, kind="ExternalInput").ap()
    bp = nc.dram_tensor("bp", [128, out_dim], F32, kind="ExternalInput").ap()
    shiftb = nc.dram_tensor("shiftb", [128, out_dim], F32, kind="ExternalInput").ap()
    iota_in = nc.dram_tensor("iota", [128, 128], BF16, kind="ExternalInput").ap()
    idxp = nc.dram_tensor("idxp", [128, 4 * nt * idxcols], I16,
                          kind="ExternalInput").ap()
    rowp = nc.dram_tensor("rowp", [128, 4 * nt * c_sub], BF16,
                          kind="ExternalInput").ap()
    valp = nc.dram_tensor("valp", [128, 4 * nt * c_sub], BF16,
                          kind="ExternalInput").ap()
    out = nc.dram_tensor("out", [nt * 128, out_dim], F32, kind="ExternalOutput").ap()
    secs = [nc.dram_tensor(f"support{s}", [sec_rows, out_dim], BF16).ap()
            for s in range(4)]

    with tile.TileContext(nc) as tc, tc.tile_pool(name="consts", bufs=1) as consts:
        w_sb = consts.tile([128, kb, out_dim], BF16)
        bp_sb = consts.tile([128, out_dim], F32)
        shift_sb = consts.tile([128, out_dim], F32)
        iota_sb = consts.tile([128, 128], BF16)
        acc = consts.tile([128, nt, out_dim], F32)
        gts = consts.tile([128, NGBUF, c_sub, out_dim], BF16)
        for i in range(NGBUF):
            nc.vector.memset(gts[:, i], 0.0)
        for k in range(kb):
            nc.sync.dma_start(w_sb[:, k, :], Wp[k * 128:(k + 1) * 128, :])
        nc.sync.dma_start(bp_sb[:], bp[:])
        nc.sync.dma_start(shift_sb[:], shiftb[:])
        nc.sync.dma_start(iota_sb[:], iota_in[:])

        # Phase-2 pools opened first: disjoint SBUF from phase-1 pools, so
        # phase-2 allocations carry no WAR deps on phase-1 releases.
        with (
            tc.tile_pool(name="slabs", bufs=2) as slabs,
            tc.tile_pool(name="sel", bufs=2) as selpool,
            tc.tile_pool(name="p2psum", bufs=6, space="PSUM") as p2psum,
        ):
            with (
                tc.tile_pool(name="xt", bufs=2) as xpool,
                tc.tile_pool(name="p1psum", bufs=2, space="PSUM") as p1psum,
                tc.tile_pool(name="p1out", bufs=4) as p1out,
            ):
                def p1_section(s4):
                    for st in range(nsup_sec):
                        gcol = (s4 * nsup_sec + st) * xcols
                        xt = xpool.tile([128, kb, xcols], BF16)
                        for k in range(kb):
                            nc.sync.dma_start(
                                xt[:, k, :],
                                xT[k * 128:(k + 1) * 128, gcol:gcol + xcols])
                        for j in range(jt):
                            ps = p1psum.tile([128, out_dim], F32)
                            for k in range(kb):
                                nc.tensor.matmul(
                                    ps[:], lhsT=xt[:, k, j * 128:(j + 1) * 128],
                                    rhs=w_sb[:, k, :],
                                    start=(k == 0), stop=(k == kb - 1))
                            so = p1out.tile([128, out_dim], BF16)
                            nc.vector.tensor_tensor(so[:], ps[:], bp_sb[:],
                                                    op=ALU.add)
                            r0 = (st * jt + j) * 128
                            nc.sync.dma_start(secs[s4][r0:r0 + 128, :], so[:])

                gbuf_i = 0

                def p2_pass(s):
                    nonlocal gbuf_i
                    for sl in range(nt // tps):
                            idx_sb = slabs.tile([128, tps * idxcols], I16, tag="idx")
                            row_sb = slabs.tile([128, tps * c_sub], BF16, tag="row")
                            val_sb = slabs.tile([128, tps * c_sub], BF16, tag="val")
                            gbase = s * nt + sl * tps
                            nc.sync.dma_start(
                                idx_sb[:], idxp[:, gbase * idxcols:(gbase + tps) * idxcols])
                            nc.sync.dma_start(
                                row_sb[:], rowp[:, gbase * c_sub:(gbase + tps) * c_sub])
                            nc.sync.dma_start(
                                val_sb[:], valp[:, gbase * c_sub:(gbase + tps) * c_sub])
                            for tt in range(tps):
                                t = sl * tps + tt
                                # sel[p, c, d] = (row[p, c] == d) * val[p, c]
                                sel = selpool.tile([128, c_sub, 128], BF16)
                                row3 = row_sb[:, tt * c_sub:(tt + 1) * c_sub].unsqueeze(2) \
                                    .to_broadcast([128, c_sub, 128])
                                val3 = val_sb[:, tt * c_sub:(tt + 1) * c_sub].unsqueeze(2) \
                                    .to_broadcast([128, c_sub, 128])
                                iota3 = iota_sb[:].unsqueeze(1) \
                                    .to_broadcast([128, c_sub, 128])
                                nc.vector.tensor_tensor(sel[:], row3, iota3, op=ALU.is_equal)
                                nc.vector.tensor_tensor(sel[:], sel[:], val3, op=ALU.mult)
                                gt = gts[:, gbuf_i % NGBUF]
                                gbuf_i += 1
                                nc.gpsimd.dma_gather(
                                    out_ap=gt[:],
                                    in_ap=secs[s][:],
                                    idxs_ap=idx_sb[:, tt * idxcols:(tt + 1) * idxcols],
                                    num_idxs=nidx,
                                    num_idxs_reg=nidx,
                                    elem_size=out_dim,
                                    single_packet=False,
                                )
                                ps = p2psum.tile([128, out_dim], F32)
                                for u in range(c_sub):
                                    nc.tensor.matmul(
                                        ps[:], lhsT=sel[:, u, :], rhs=gt[:, u, :],
                                        start=(u == 0), stop=(u == c_sub - 1))
                                if s == 0:
                                    nc.vector.tensor_copy(acc[:, t, :], ps[:])
                                elif s < 3:
                                    nc.vector.tensor_tensor(acc[:, t, :], acc[:, t, :],
                                                            ps[:], op=ALU.add)
                                else:
                                    # final section: fuse BN shift + GELU + store
                                    ob = selpool.tile([128, out_dim], F32, tag="ob")
                                    nc.vector.tensor_tensor(ob[:], acc[:, t, :],
                                                            ps[:], op=ALU.add)
                                    ob2 = selpool.tile([128, out_dim], F32, tag="ob2")
                                    nc.vector.tensor_tensor(ob2[:], ob[:],
                                                            shift_sb[:], op=ALU.add)
                                    ob3 = selpool.tile([128, out_dim], F32, tag="ob3")
                                    nc.scalar.activation(ob3[:], ob2[:], AF.Gelu)
                                    nc.sync.dma_start(out[t * 128:(t + 1) * 128, :],
                                                      ob3[:])

                p1_section(0)
                p1_section(1)
                p2_pass(0)
                p1_section(2)
                p2_pass(1)
                p1_section(3)
                p2_pass(2)
                p2_pass(3)

    nc.compile()
    return nc


def _preprocess(x, edge_row, edge_col, edge_val, W, b, gamma, beta,
                running_mean, running_var, bn_eps=1e-5):
    n, in_dim = x.shape
    out_dim = W.shape[1]
    npad = ((n + 4 * XCOLS - 1) // (4 * XCOLS)) * (4 * XCOLS)
    sec_rows = npad // 4
    assert sec_rows <= 32768, "support section must be int16-addressable"
    shard = n // N_CORES
    assert shard * N_CORES == n
    nt = (shard + 127) // 128
    nt = ((nt + TPS - 1) // TPS) * TPS

    inv_std = 1.0 / np.sqrt(running_var.astype(np.float64) + bn_eps)
    scale = (inv_std * gamma.astype(np.float64)).astype(np.float32)
    shift = (beta.astype(np.float64) - running_mean.astype(np.float64) * inv_std
             * gamma.astype(np.float64)).astype(np.float32)

    xT = np.zeros((in_dim, npad), BF)
    xT[:, :n] = np.ascontiguousarray(x.T).astype(BF)
    Wp = (W * scale[None, :]).astype(BF)
    bp = np.ascontiguousarray(
        np.broadcast_to((b * scale).astype(np.float32), (128, out_dim)))
    shiftb = np.ascontiguousarray(np.broadcast_to(shift, (128, out_dim)))
    iota = np.ascontiguousarray(
        np.broadcast_to(np.arange(128, dtype=np.float32), (128, 128))).astype(BF)

    per_core = []
    c_sub = 1
    ng = 4 * nt
    for m in range(N_CORES):
        lo, hi = m * shard, (m + 1) * shard
        mask = (edge_row >= lo) & (edge_row < hi)
        er = (edge_row[mask] - lo).astype(np.int64)
        ec = edge_col[mask].astype(np.int64)
        ev = edge_val[mask].astype(np.float32)
        tile_of = er >> 7
        sec_of = ec // sec_rows
        gid = sec_of * nt + tile_of
        order = np.argsort(gid, kind="stable")
        er, ec, ev, gid = er[order], ec[order], ev[order], gid[order]
        counts = np.bincount(gid, minlength=ng)
        per_core.append((er, ec, ev, gid, counts))
        c_sub = max(c_sub, int(((counts + 127) // 128).max()))
    nidx = c_sub * 128
    idxcols = nidx // 16

    in_maps = []
    for m in range(N_CORES):
        er, ec, ev, gid, counts = per_core[m]
        starts = np.zeros(ng, np.int64)
        np.cumsum(counts[:-1], out=starts[1:])
        rank = np.arange(len(er)) - starts[gid]
        rowp = np.zeros((128, ng * c_sub), BF)
        valp = np.zeros((128, ng * c_sub), BF)
        rowp[rank & 127, gid * c_sub + (rank >> 7)] = (er & 127).astype(BF)
        valp[rank & 127, gid * c_sub + (rank >> 7)] = ev.astype(BF)
        # constant full index count: padding entries gather row 0, val 0
        idx16 = np.zeros((16, ng * idxcols), np.int16)
        idx16[rank & 15, gid * idxcols + (rank >> 4)] = \
            (ec % sec_rows).astype(np.int16)
        in_maps.append({
            "xT": xT, "Wp": Wp, "bp": bp, "shiftb": shiftb, "iota": iota,
            "idxp": np.ascontiguousarray(np.tile(idx16, (8, 1))),
            "rowp": np.ascontiguousarray(rowp),
            "valp": np.ascontiguousarray(valp),
        })

    params = dict(in_dim=in_dim, out_dim=out_dim, npad=npad,
                  nt=nt, c_sub=c_sub, tps=TPS, xcols=XCOLS)
    return in_maps, params, shard


def kernel(x, edge_row, edge_col, edge_val, W, b, gamma, beta,
           running_mean, running_var):
    x = np.asarray(x)
    edge_row = np.asarray(edge_row)
    edge_col = np.asarray(edge_col)
    edge_val = np.asarray(edge_val)
    W = np.asarray(W)
    b = np.asarray(b)
    gamma = np.asarray(gamma)
    beta = np.asarray(beta)
    running_mean = np.asarray(running_mean)
    running_var = np.asarray(running_var)

    in_maps, params, shard = _preprocess(
        x, edge_row, edge_col, edge_val, W, b, gamma, beta,
        running_mean, running_var)
    nc = _build_program(**params)
    res = run_bass_kernel_spmd(nc, in_maps, core_ids=list(range(N_CORES)))
    outs = [res.results[m]["out"][:shard] for m in range(N_CORES)]
    return np.concatenate(outs, axis=0).astype(np.float32)
